# revision 1
# baseline (speedup 1.0000x reference)
"""MoE router kernel for Trainium2 (8 NeuronCores, SPMD over the token dim).

Computes, for hidden_states [8192, 4096] f32, W [8, 4096], b [8]:
  router_logits    [8192, 8] f32   = h @ W.T + b
  router_weights   [8192, 2] f32   = renormalized top-2 softmax probs
  selected_experts [8192, 2] int32 = top-2 expert indices
  expert_mask      [8, 2, 8192] i32= one_hot(selected_experts).transpose(2,1,0)

Sharding: token dim split 8 ways (1024 tokens/core); gate weight replicated.
All matmul math on-device in f32 (PE transpose of hidden chunks + PSUM
accumulation over the 4096-dim contraction in 32 chunks of 128).
"""
import numpy as np
from contextlib import ExitStack

import concourse.bass as bass
import concourse.mybir as mybir
import concourse.tile as tile
from concourse import bacc
from concourse.masks import make_identity
from concourse.bass_utils import run_bass_kernel_spmd

T = 8192
D = 4096
E = 8
NCORES = 8
TPC = T // NCORES          # tokens per core (1024)
NTILE = TPC // 128         # token tiles per core (8)
NCHUNK = D // 128          # contraction chunks (32)
NGROUP = 8                 # transpose groups per tile (4 chunks each)
BIG = 100.0

f32 = mybir.dt.float32
i32 = mybir.dt.int32
AX = mybir.AxisListType.X
OP = mybir.AluOpType


def build():
    nc = bacc.Bacc("TRN2", target_bir_lowering=False, debug=False,
                   num_devices=NCORES)
    h_ext = nc.declare_dram_parameter("h", [TPC, D], f32, isOutput=False)
    wt_ext = nc.declare_dram_parameter("wt", [D, E], f32, isOutput=False)
    bias_ext = nc.declare_dram_parameter("bias", [128, E], f32, isOutput=False)
    iota_ext = nc.declare_dram_parameter("iota", [128, E], f32, isOutput=False)
    iotab_ext = nc.declare_dram_parameter("iotab", [128, E], f32, isOutput=False)
    lg_ext = nc.declare_dram_parameter("logits", [TPC, E], f32, isOutput=True)
    w_ext = nc.declare_dram_parameter("weights", [TPC, 2], f32, isOutput=True)
    sel_ext = nc.declare_dram_parameter("sel", [TPC, 2], i32, isOutput=True)
    mask_ext = nc.declare_dram_parameter("mask", [2 * E, TPC], i32, isOutput=True)

    with tile.TileContext(nc) as tc, ExitStack() as ctx:
        p_h = ctx.enter_context(tc.tile_pool(name="h", bufs=2))
        p_hts = ctx.enter_context(tc.tile_pool(name="hts", bufs=2))
        p_const = ctx.enter_context(tc.tile_pool(name="const", bufs=1))
        p_sm = ctx.enter_context(tc.tile_pool(name="sm", bufs=2))
        p_macc = ctx.enter_context(tc.tile_pool(name="macc", bufs=1))
        p_pt = ctx.enter_context(tc.tile_pool(name="pt", bufs=3, space="PSUM"))
        p_lp = ctx.enter_context(tc.tile_pool(name="lp", bufs=2, space="PSUM"))
        p_mt = ctx.enter_context(tc.tile_pool(name="mt", bufs=2, space="PSUM"))

        ident = p_const.tile([128, 128], f32, tag="ident")
        make_identity(nc, ident)
        wt_sb = p_const.tile([128, NCHUNK, E], f32, tag="wt")
        nc.gpsimd.dma_start(wt_sb[:], wt_ext.rearrange("(c p) e -> p c e", p=128))
        bias_sb = p_const.tile([128, E], f32, tag="bias")
        nc.gpsimd.dma_start(bias_sb[:], bias_ext[:])
        iota_sb = p_const.tile([128, E], f32, tag="iota")
        nc.gpsimd.dma_start(iota_sb[:], iota_ext[:])
        iotab_sb = p_const.tile([128, E], f32, tag="iotab")
        nc.gpsimd.dma_start(iotab_sb[:], iotab_ext[:])

        mask_acc = p_macc.tile([2 * E, TPC], i32, tag="macc")

        # software-pipelined: mask transpose of tile i-1 is emitted between
        # the transpose and matmul phases of tile i to keep PE fed
        pending_mask = None  # (mask_sb, tile_idx)

        def emit_mask_transpose():
            nonlocal pending_mask
            if pending_mask is None:
                return
            m_sb, ti = pending_mask
            m_ps = p_mt.tile([2 * E, 128], f32, tag="mps")
            nc.tensor.transpose(m_ps[:], m_sb[:], ident[:])
            nc.vector.tensor_copy(mask_acc[:, ti * 128:(ti + 1) * 128], m_ps[:])
            pending_mask = None

        for i in range(NTILE):
            h_sb = p_h.tile([128, D], f32, tag="h")
            nc.sync.dma_start(h_sb[:], h_ext[i * 128:(i + 1) * 128, :])

            # phase 1: transpose 32 d-chunks via PE, 4 per PSUM bank, copy to SBUF
            hts_sb = p_hts.tile([128, D], f32, tag="hts")
            for g in range(NGROUP):
                ht_ps = p_pt.tile([128, 512], f32, tag="pt")
                for j in range(4):
                    c = g * 4 + j
                    nc.tensor.transpose(
                        ht_ps[:, j * 128:(j + 1) * 128],
                        h_sb[:, c * 128:(c + 1) * 128], ident[:])
                eng = nc.vector if g % 2 == 0 else nc.scalar
                if g % 2 == 0:
                    nc.vector.tensor_copy(
                        hts_sb[:, g * 512:(g + 1) * 512], ht_ps[:])
                else:
                    nc.scalar.copy(hts_sb[:, g * 512:(g + 1) * 512], ht_ps[:])

            emit_mask_transpose()

            # phase 2: 32 accumulating matmuls -> logits psum [128 tok, 8 exp]
            lg_ps = p_lp.tile([128, E], f32, tag="lp")
            for c in range(NCHUNK):
                nc.tensor.matmul(
                    lg_ps[:], hts_sb[:, c * 128:(c + 1) * 128], wt_sb[:, c, :],
                    start=(c == 0), stop=(c == NCHUNK - 1))

            # phase 3: bias add + top-2 + weights + one-hot mask (DVE/ACT)
            lg_sb = p_sm.tile([128, E], f32, tag="lg")
            nc.vector.tensor_add(lg_sb[:], lg_ps[:], bias_sb[:])
            nc.gpsimd.dma_start(lg_ext[i * 128:(i + 1) * 128, :], lg_sb[:])

            mx = p_sm.tile([128, 8], f32, tag="mx")
            nc.vector.max(out=mx[:], in_=lg_sb[:])
            v0 = mx[:, 0:1]
            v1 = mx[:, 1:2]

            idxf = p_sm.tile([128, 2], f32, tag="idxf")
            eq = p_sm.tile([128, E], f32, tag="eq")
            mi = p_sm.tile([128, E], f32, tag="mi")
            # idx0: min index where logits == v0
            nc.vector.tensor_scalar(eq[:], lg_sb[:], v0, None, op0=OP.is_equal)
            nc.vector.tensor_scalar(mi[:], eq[:], -BIG, None, op0=OP.mult)
            nc.vector.tensor_add(mi[:], mi[:], iotab_sb[:])
            nc.vector.tensor_reduce(idxf[:, 0:1], mi[:], axis=AX, op=OP.min)
            # idx1: min index where logits == v1 and index != idx0
            neq = p_sm.tile([128, E], f32, tag="neq")
            nc.vector.tensor_scalar(eq[:], lg_sb[:], v1, None, op0=OP.is_equal)
            nc.vector.tensor_scalar(neq[:], iota_sb[:], idxf[:, 0:1], None,
                                    op0=OP.not_equal)
            nc.vector.tensor_mul(eq[:], eq[:], neq[:])
            nc.vector.tensor_scalar(mi[:], eq[:], -BIG, None, op0=OP.mult)
            nc.vector.tensor_add(mi[:], mi[:], iotab_sb[:])
            nc.vector.tensor_reduce(idxf[:, 1:2], mi[:], axis=AX, op=OP.min)

            sel_sb = p_sm.tile([128, 2], i32, tag="sel")
            nc.vector.tensor_copy(sel_sb[:], idxf[:])
            nc.gpsimd.dma_start(sel_ext[i * 128:(i + 1) * 128, :], sel_sb[:])

            # weights: w0 = 1/(1+exp(v1-v0)), w1 = exp(v1-v0)*w0
            w_sb = p_sm.tile([128, 2], f32, tag="w")
            dd = p_sm.tile([128, 1], f32, tag="dd")
            ee = p_sm.tile([128, 1], f32, tag="ee")
            ss = p_sm.tile([128, 1], f32, tag="ss")
            nc.vector.tensor_sub(dd[:], v1, v0)
            nc.scalar.activation(ee[:], dd[:], mybir.ActivationFunctionType.Exp)
            nc.vector.tensor_scalar_add(ss[:], ee[:], 1.0)
            nc.vector.reciprocal(w_sb[:, 0:1], ss[:])
            nc.vector.tensor_mul(w_sb[:, 1:2], ee[:], w_sb[:, 0:1])
            nc.gpsimd.dma_start(w_ext[i * 128:(i + 1) * 128, :], w_sb[:])

            # one-hot mask [128 tok, 8 exp, 2 k] (transposed to [16, t] later)
            m_sb = p_sm.tile([128, E, 2], f32, tag="msb")
            nc.vector.tensor_scalar(m_sb[:, :, 0], iota_sb[:], idxf[:, 0:1],
                                    None, op0=OP.is_equal)
            nc.vector.tensor_scalar(m_sb[:, :, 1], iota_sb[:], idxf[:, 1:2],
                                    None, op0=OP.is_equal)
            pending_mask = (m_sb, i)

        emit_mask_transpose()
        nc.gpsimd.dma_start(mask_ext[:], mask_acc[:])

    nc.finalize()
    return nc


_NC = None


def _get_nc():
    global _NC
    if _NC is None:
        _NC = build()
    return _NC


def kernel(hidden_states: np.ndarray, W: np.ndarray, b: np.ndarray):
    hidden_states = np.ascontiguousarray(hidden_states, dtype=np.float32)
    W = np.asarray(W, dtype=np.float32)
    b = np.asarray(b, dtype=np.float32)

    wt = np.ascontiguousarray(W.T)                              # [4096, 8]
    bias_rep = np.ascontiguousarray(np.tile(b[None, :], (128, 1)))
    iota = np.tile(np.arange(E, dtype=np.float32)[None, :], (128, 1))
    iota = np.ascontiguousarray(iota)
    iotab = np.ascontiguousarray(iota + BIG)

    h_sh = hidden_states.reshape(NCORES, TPC, D)
    in_maps = [
        {"h": np.ascontiguousarray(h_sh[c]), "wt": wt, "bias": bias_rep,
         "iota": iota, "iotab": iotab}
        for c in range(NCORES)
    ]
    nc = _get_nc()
    res = run_bass_kernel_spmd(nc, in_maps, core_ids=list(range(NCORES)))
    rs = res.results

    router_logits = np.concatenate([r["logits"] for r in rs], axis=0)
    router_weights = np.concatenate([r["weights"] for r in rs], axis=0)
    selected_experts = np.concatenate([r["sel"] for r in rs], axis=0)
    mask = np.concatenate([r["mask"] for r in rs], axis=1)      # [16, 8192]
    expert_mask = np.ascontiguousarray(mask.reshape(E, 2, T))

    return (router_logits, router_weights, selected_experts, expert_mask)


# revision 4
# speedup vs baseline: 62.7922x; 62.7922x over previous
"""MoE router kernel for Trainium2 (8 NeuronCores, SPMD over the token dim).

Computes, for hidden_states [8192, 4096] f32, W [8, 4096], b [8]:
  router_logits    [8192, 8] f32   = h @ W.T + b
  router_weights   [8192, 2] f32   = renormalized top-2 softmax probs
  selected_experts [8192, 2] int32 = top-2 expert indices
  expert_mask      [8, 2, 8192] i32= one_hot(selected_experts).transpose(2,1,0)

Sharding: token dim split 8 ways (1024 tokens/core); gate weight replicated.
All matmul math on-device in f32 (PE transpose of hidden chunks + PSUM
accumulation over the 4096-dim contraction in 32 chunks of 128).
"""
import numpy as np
from contextlib import ExitStack

import concourse.bass as bass
import concourse.mybir as mybir
import concourse.tile as tile
from concourse import bacc
from concourse.masks import make_identity
from concourse.bass_utils import run_bass_kernel_spmd

T = 8192
D = 4096
E = 8
NCORES = 8
TPC = T // NCORES          # tokens per core (1024)
NTILE = TPC // 128         # token tiles per core (8)
NCHUNK = D // 128          # contraction chunks (32)
NGROUP = 8                 # transpose groups per tile (4 chunks each)
BIG = 100.0

f32 = mybir.dt.float32
i32 = mybir.dt.int32
AX = mybir.AxisListType.X
OP = mybir.AluOpType


def build(reps: int = 1):
    nc = bacc.Bacc("TRN2", target_bir_lowering=False, debug=False,
                   num_devices=NCORES)
    h_ext = nc.declare_dram_parameter("h", [TPC, D], f32, isOutput=False)
    wt_ext = nc.declare_dram_parameter("wt", [D, E], f32, isOutput=False)
    bias_ext = nc.declare_dram_parameter("bias", [128, E], f32, isOutput=False)
    iota_ext = nc.declare_dram_parameter("iota", [128, E], f32, isOutput=False)
    iotab_ext = nc.declare_dram_parameter("iotab", [128, E], f32, isOutput=False)
    lg_ext = nc.declare_dram_parameter("logits", [TPC, E], f32, isOutput=True)
    w_ext = nc.declare_dram_parameter("weights", [TPC, 2], f32, isOutput=True)
    sel_ext = nc.declare_dram_parameter("sel", [TPC, 2], i32, isOutput=True)
    mask_ext = nc.declare_dram_parameter("mask", [2 * E, TPC], i32, isOutput=True)

    with tile.TileContext(nc) as tc, ExitStack() as ctx:
        p_h = ctx.enter_context(tc.tile_pool(name="h", bufs=2))
        p_hts = ctx.enter_context(tc.tile_pool(name="hts", bufs=2))
        p_const = ctx.enter_context(tc.tile_pool(name="const", bufs=1))
        p_sm = ctx.enter_context(tc.tile_pool(name="sm", bufs=2))
        p_macc = ctx.enter_context(tc.tile_pool(name="macc", bufs=1))
        p_pt = ctx.enter_context(tc.tile_pool(name="pt", bufs=3, space="PSUM"))
        p_lp = ctx.enter_context(tc.tile_pool(name="lp", bufs=2, space="PSUM"))
        p_mt = ctx.enter_context(tc.tile_pool(name="mt", bufs=2, space="PSUM"))

        ident = p_const.tile([128, 128], f32, tag="ident")
        make_identity(nc, ident)
        wt_sb = p_const.tile([128, NCHUNK, E], f32, tag="wt")
        nc.gpsimd.dma_start(wt_sb[:], wt_ext.rearrange("(c p) e -> p c e", p=128))
        bias_sb = p_const.tile([128, E], f32, tag="bias")
        nc.gpsimd.dma_start(bias_sb[:], bias_ext[:])
        iota_sb = p_const.tile([128, E], f32, tag="iota")
        nc.gpsimd.dma_start(iota_sb[:], iota_ext[:])
        iotab_sb = p_const.tile([128, E], f32, tag="iotab")
        nc.gpsimd.dma_start(iotab_sb[:], iotab_ext[:])

        mask_acc = p_macc.tile([2 * E, TPC], i32, tag="macc")

        loop_cm = tc.For_i(0, reps, 1) if reps > 1 else None
        if loop_cm is not None:
            loop_cm.__enter__()

        # software-pipelined: mask transpose of tile i-1 is emitted between
        # the transpose and matmul phases of tile i to keep PE fed
        pending_mask = None  # (mask_sb, tile_idx)

        def emit_mask_transpose():
            nonlocal pending_mask
            if pending_mask is None:
                return
            m_sb, ti = pending_mask
            m_ps = p_mt.tile([2 * E, 128], f32, tag="mps")
            nc.tensor.transpose(m_ps[:], m_sb[:], ident[:])
            nc.vector.tensor_copy(mask_acc[:, ti * 128:(ti + 1) * 128], m_ps[:])
            pending_mask = None

        for i in range(NTILE):
            h_sb = p_h.tile([128, D], f32, tag="h")
            nc.sync.dma_start(h_sb[:], h_ext[i * 128:(i + 1) * 128, :])

            # phase 1: transpose 32 d-chunks via PE, 4 per PSUM bank, copy to SBUF
            hts_sb = p_hts.tile([128, D], f32, tag="hts")
            for g in range(NGROUP):
                ht_ps = p_pt.tile([128, 512], f32, tag="pt")
                for j in range(4):
                    c = g * 4 + j
                    nc.tensor.transpose(
                        ht_ps[:, j * 128:(j + 1) * 128],
                        h_sb[:, c * 128:(c + 1) * 128], ident[:])
                eng = nc.vector if g % 2 == 0 else nc.scalar
                if g % 2 == 0:
                    nc.vector.tensor_copy(
                        hts_sb[:, g * 512:(g + 1) * 512], ht_ps[:])
                else:
                    nc.scalar.copy(hts_sb[:, g * 512:(g + 1) * 512], ht_ps[:])

            emit_mask_transpose()

            # phase 2: 32 accumulating matmuls -> logits psum [128 tok, 8 exp]
            lg_ps = p_lp.tile([128, E], f32, tag="lp")
            for c in range(NCHUNK):
                nc.tensor.matmul(
                    lg_ps[:], hts_sb[:, c * 128:(c + 1) * 128], wt_sb[:, c, :],
                    start=(c == 0), stop=(c == NCHUNK - 1))

            # phase 3: bias add + top-2 + weights + one-hot mask (DVE/ACT)
            lg_sb = p_sm.tile([128, E], f32, tag="lg")
            nc.vector.tensor_add(lg_sb[:], lg_ps[:], bias_sb[:])
            nc.gpsimd.dma_start(lg_ext[i * 128:(i + 1) * 128, :], lg_sb[:])

            mx = p_sm.tile([128, 8], f32, tag="mx")
            nc.vector.max(out=mx[:], in_=lg_sb[:])
            v0 = mx[:, 0:1]
            v1 = mx[:, 1:2]

            idxf = p_sm.tile([128, 2], f32, tag="idxf")
            eq = p_sm.tile([128, E], f32, tag="eq")
            mi = p_sm.tile([128, E], f32, tag="mi")
            # idx0: min index where logits == v0
            nc.vector.tensor_scalar(eq[:], lg_sb[:], v0, None, op0=OP.is_equal)
            nc.vector.tensor_scalar(mi[:], eq[:], -BIG, None, op0=OP.mult)
            nc.vector.tensor_add(mi[:], mi[:], iotab_sb[:])
            nc.vector.tensor_reduce(idxf[:, 0:1], mi[:], axis=AX, op=OP.min)
            # idx1: min index where logits == v1 and index != idx0
            neq = p_sm.tile([128, E], f32, tag="neq")
            nc.vector.tensor_scalar(eq[:], lg_sb[:], v1, None, op0=OP.is_equal)
            nc.vector.tensor_scalar(neq[:], iota_sb[:], idxf[:, 0:1], None,
                                    op0=OP.not_equal)
            nc.vector.tensor_mul(eq[:], eq[:], neq[:])
            nc.vector.tensor_scalar(mi[:], eq[:], -BIG, None, op0=OP.mult)
            nc.vector.tensor_add(mi[:], mi[:], iotab_sb[:])
            nc.vector.tensor_reduce(idxf[:, 1:2], mi[:], axis=AX, op=OP.min)

            sel_sb = p_sm.tile([128, 2], i32, tag="sel")
            nc.vector.tensor_copy(sel_sb[:], idxf[:])
            nc.gpsimd.dma_start(sel_ext[i * 128:(i + 1) * 128, :], sel_sb[:])

            # weights: w0 = 1/(1+exp(v1-v0)), w1 = exp(v1-v0)*w0
            w_sb = p_sm.tile([128, 2], f32, tag="w")
            dd = p_sm.tile([128, 1], f32, tag="dd")
            ee = p_sm.tile([128, 1], f32, tag="ee")
            ss = p_sm.tile([128, 1], f32, tag="ss")
            nc.vector.tensor_sub(dd[:], v1, v0)
            nc.scalar.activation(ee[:], dd[:], mybir.ActivationFunctionType.Exp)
            nc.vector.tensor_scalar_add(ss[:], ee[:], 1.0)
            nc.vector.reciprocal(w_sb[:, 0:1], ss[:])
            nc.vector.tensor_mul(w_sb[:, 1:2], ee[:], w_sb[:, 0:1])
            nc.gpsimd.dma_start(w_ext[i * 128:(i + 1) * 128, :], w_sb[:])

            # one-hot mask [128 tok, 8 exp, 2 k] (transposed to [16, t] later)
            m_sb = p_sm.tile([128, E, 2], f32, tag="msb")
            nc.vector.tensor_scalar(m_sb[:, :, 0], iota_sb[:], idxf[:, 0:1],
                                    None, op0=OP.is_equal)
            nc.vector.tensor_scalar(m_sb[:, :, 1], iota_sb[:], idxf[:, 1:2],
                                    None, op0=OP.is_equal)
            pending_mask = (m_sb, i)

        emit_mask_transpose()
        if loop_cm is not None:
            loop_cm.__exit__(None, None, None)
        nc.gpsimd.dma_start(mask_ext[:], mask_acc[:])

    nc.finalize()
    return nc


_NC = None


def _get_nc():
    global _NC
    if _NC is None:
        _NC = build()
    return _NC


def kernel(hidden_states: np.ndarray, W: np.ndarray, b: np.ndarray):
    hidden_states = np.ascontiguousarray(hidden_states, dtype=np.float32)
    W = np.asarray(W, dtype=np.float32)
    b = np.asarray(b, dtype=np.float32)

    wt = np.ascontiguousarray(W.T)                              # [4096, 8]
    bias_rep = np.ascontiguousarray(np.tile(b[None, :], (128, 1)))
    iota = np.tile(np.arange(E, dtype=np.float32)[None, :], (128, 1))
    iota = np.ascontiguousarray(iota)
    iotab = np.ascontiguousarray(iota + BIG)

    h_sh = hidden_states.reshape(NCORES, TPC, D)
    in_maps = [
        {"h": np.ascontiguousarray(h_sh[c]), "wt": wt, "bias": bias_rep,
         "iota": iota, "iotab": iotab}
        for c in range(NCORES)
    ]
    nc = _get_nc()
    res = run_bass_kernel_spmd(nc, in_maps, core_ids=list(range(NCORES)))
    rs = res.results

    router_logits = np.concatenate([r["logits"] for r in rs], axis=0)
    router_weights = np.concatenate([r["weights"] for r in rs], axis=0)
    selected_experts = np.concatenate([r["sel"] for r in rs], axis=0)
    mask = np.concatenate([r["mask"] for r in rs], axis=1)      # [16, 8192]
    expert_mask = np.ascontiguousarray(mask.reshape(E, 2, T))

    return (router_logits, router_weights, selected_experts, expert_mask)
